# revision 1
# baseline (speedup 1.0000x reference)
"""Multi-head attention Bass/Tile kernel for 8 TRN2 NeuronCores.

Problem: nn_MultiHeadAttention (B=4, T1=T2=2048, d_model=256, d_key=32, H=8,
per-head value dim = d_model).  Reference math (no score scaling, no mask):

    k = key   @ WK^T + bk           [B, T1, 256]   (head h -> cols 32h..32h+32)
    q = query @ WQ^T + bq           [B, T2, 256]
    v = value @ WV^T + bv           [B, T1, 2048]  (head h -> cols 256h..256h+256)
    scores_h = k_h q_h^T            [T1, T2]
    attn = softmax over T1 (keys)
    emb_h = attn^T v_h              [T2, 256]
    out = emb' @ WO^T + bo          emb' channel c = d*8 + h (d outer, h inner)

Sharding: core c handles (batch b = c//2, query half qs = c%2) -> each core
computes the full output slice out[b, qs*1024:(qs+1)*1024, :].  No collectives.

Per-core algorithm (bf16 matmuls + XBAR transposes + 1-cycle denominators):
  - fp32 inputs DMA'd in rolling chunks, cast to bf16 on Pool/DVE, transposed
    to channel-major via the DMA XBAR transpose (dma_start_transpose), not the
    PE.  wo alone uses PE transposes (head-permuted columns aren't XBAR-able).
  - k/q projections write bf16 via ACT (+bias).  All attention matmuls are
    bf16: fp8 on any attention operand (k, q, val or wv) costs 2.6-5% output
    error (quantization of the query/weight side doesn't average out across
    keys, and exp amplifies score noise), blowing the 2e-2 budget.
  - E = exp(scores) on ACT in [s, q] layout (bf16 range; no max subtraction).
  - numer_h[d,q] = v_h^T E on PE; v bias is folded into the final output
    bias (sum_h bv_h @ WO_h^T + wo_b, computed on-device) since softmax
    weights sum to 1.
  - denominators: matmul with lhsT = E-tile (stationary), rhs = ones[128,1]
    -> out[128q, 1] accumulated over s-tiles.  Cost scales with out free
    size, so each matmul is ~1 cycle, and the result lands directly in
    [q-part] layout for the reciprocal.
  - WO per head pair fused with 1/denom via scalar_tensor_tensor (+bias).
  - Software pipeline: unit u = (head-pair, q-chunk); scores+exp of unit u
    are emitted head-major with the denom/vproj/numer work of unit u-2
    interleaved between score groups, so the PE always has exp-independent
    work behind each ACT-paced score tile.  Staging pools are split by
    lifetime so late main tensors reuse their SBUF without stalling.

kernel(**inputs) takes the FULL unsharded inputs and returns the full output.
"""

import numpy as np
from contextlib import ExitStack

import concourse.bass as bass
import concourse.bacc as bacc
import concourse.mybir as mybir
import concourse.tile as tile
import concourse.bass_isa as bass_isa
from concourse.bass_utils import run_bass_kernel_spmd
from concourse.masks import make_identity

P = 128
B, T1, T2, DM, DK, H = 4, 2048, 2048, 256, 32, 8
QSH = T2 // 2  # queries per core
N_CORES = 8

F32 = mybir.dt.float32
BF16 = mybir.dt.bfloat16
FP8 = mybir.dt.float8e4
AF = mybir.ActivationFunctionType
DR = mybir.MatmulPerfMode.DoubleRow

ST = T1 // P        # 16 key/seq tiles
QT = QSH // P       # 8 query tiles per core
QC = 512            # query chunk (PSUM free dim)
NQC = QSH // QC     # 2 query chunks


def _dup2(ap):
    """[p, n] AP -> [p, 2(stride 0), n]: duplicates the free dim for DR."""
    return bass.AP(tensor=ap.tensor, offset=ap.offset,
                   ap=[list(ap.ap[0]), [0, 2], list(ap.ap[-1])])


def _build_bass():
    nc = bacc.Bacc("TRN2", target_bir_lowering=False, debug=False)

    key = nc.dram_tensor("key_x", [T1, DM], F32, kind="ExternalInput").ap()
    qry = nc.dram_tensor("qry_x", [QSH, DM], F32, kind="ExternalInput").ap()
    val = nc.dram_tensor("val_x", [T1, DM], F32, kind="ExternalInput").ap()
    wk = nc.dram_tensor("wk", [DM, DM], F32, kind="ExternalInput").ap()
    wkb = nc.dram_tensor("wkb", [DM], F32, kind="ExternalInput").ap()
    wq = nc.dram_tensor("wq", [DM, DM], F32, kind="ExternalInput").ap()
    wqb = nc.dram_tensor("wqb", [DM], F32, kind="ExternalInput").ap()
    wv = nc.dram_tensor("wv", [H * DM, DM], F32, kind="ExternalInput").ap()
    wvb = nc.dram_tensor("wvb", [H * DM], F32, kind="ExternalInput").ap()
    wo = nc.dram_tensor("wo", [DM, H * DM], F32, kind="ExternalInput").ap()
    wob = nc.dram_tensor("wob", [DM], F32, kind="ExternalInput").ap()
    out = nc.dram_tensor("out_y", [QSH, DM], F32, kind="ExternalOutput").ap()

    with tile.TileContext(nc, pool_alloc_mode="queue") as tc:
        with ExitStack() as ctx:
            _body(ctx, tc, key, qry, val, wk, wkb, wq, wqb, wv, wvb, wo, wob, out)
    nc.compile()
    return nc


def _body(ctx, tc, key, qry, val, wk, wkb, wq, wqb, wv, wvb, wo, wob, out):
    nc = tc.nc
    consts = ctx.enter_context(tc.tile_pool(name="consts", bufs=1))
    main = ctx.enter_context(tc.tile_pool(name="main", bufs=1))
    # One PSUM pool, 8 banks total:
    #   SC: [128,2,512] f32 x2   (4 banks) score tiles / wo-transpose psums
    #   NU: [128,512]  f32 x2    (2 banks) k/q proj, numer accum, WO psums
    #   VP: [128,512]  f32 x2    (2 banks) v-proj, denominator chains
    pP = ctx.enter_context(tc.tile_pool(name="pP", bufs=1, space="PSUM"))

    ident_bf = consts.tile([P, P], BF16)
    make_identity(nc, ident_bf)
    # touch the ACT function table early so its load is off the critical path
    actwarm = consts.tile([1, 1], F32)
    nc.scalar.activation(out=actwarm, in_=actwarm, func=AF.Exp)
    ones_bf = consts.tile([P, 1], BF16)
    nc.vector.memset(ones_bf, 1.0)

    # biases (loaded during stage 0)
    wk_b = consts.tile([P, 2], F32)
    wq_b = consts.tile([P, 2], F32)
    # bv is folded into the final bias: out += (sum_h bv_h @ WO_h^T) + wo_b
    # bv_t[p, kt] = wvb[kt*128+p] (kt = 2h+db matches woTp's row tiling)
    bv_t = consts.tile([P, 16], F32)
    bv_bf = consts.tile([P, 16], BF16)
    wob_row = consts.tile([1, DM], F32)
    bo_row = consts.tile([1, DM], F32)
    wob_bc = consts.tile([P, DM], F32)

    # persistent channel-major tensors -- created (= address-reserved) BEFORE
    # the staging pools so the main loop never waits on staging teardown
    # kT[p, hg, s]: head h = 4*hg + p//32, channel feat = p%32 (bf16: fp8
    # anywhere in the attention path costs >2% output error)
    kT = main.tile([P, 2, T1], BF16)
    qT = main.tile([P, 2, QSH], BF16)
    valT = main.tile([P, ST // 4, 8, P], BF16)   # [d, s] grouped (g, t=2*(st%4)+dt, c)
    wvT4 = main.tile([P, 4, 8, P], BF16)         # [d, c] grouped (g, t=2*(cst%4)+dt, cc)
    woTp = main.tile([P, 16, DM], BF16)          # [c'=h*256+d, cout]
    wo_bf = main.tile([P, 2, H * DM], BF16)
    # E tiles: manual 5-slot ring, pre-reserved; slot (2u+hh) % 5
    E_ring = [main.tile([P, ST, QC], BF16, name=f"Ering{i}") for i in range(5)]

    # ---------------- stage 0: load + cast + XBAR transpose + k/q proj ------
    # Two staging pools split by lifetime: the k/q staging dies ~25us in,
    # the v/wo staging ~50us; late main tensors reuse their space in order.
    with ExitStack() as s0:
        stgK = s0.enter_context(tc.tile_pool(name="stgK", bufs=1))
        stgV = s0.enter_context(tc.tile_pool(name="stgV", bufs=1))

        def load(pool, src_ap, n_units, label, unit=DM, bufs=2):
            src = src_ap.rearrange("(n p) d -> p n d", p=P)
            chunk = 4 if unit == DM else 1
            tiles = []
            for i in range(0, n_units, chunk):
                j = min(n_units, i + chunk)
                f = pool.tile([P, chunk, unit], F32, tag=f"ld_{label}",
                              name=f"ld_{label}_{i}", bufs=bufs)
                nc.sync.dma_start(out=f[:, :j - i, :], in_=src[:, i:j, :])
                tiles.append(f)
            return tiles

        ld_key = load(stgK, key, ST, "key", bufs=2)
        ld_wk = load(stgK, wk, 2, "wk", bufs=1)
        ld_wq = load(stgK, wq, 2, "wq", bufs=1)
        ld_qry = load(stgK, qry, QT, "qry", bufs=1)
        ld_val = load(stgV, val, ST, "val", bufs=2)
        ld_wv = load(stgV, wv, ST, "wv", bufs=2)
        wo_src = wo.rearrange("(n p) (h d2) -> p n h d2", p=P, h=2)
        ld_wo = []
        for t in range(2):
            for hf in range(2):
                f = stgV.tile([P, 1, H * DM // 2], F32, tag="ld_wo",
                              name=f"ld_wo_{t}_{hf}", bufs=1)
                nc.sync.dma_start(out=f, in_=wo_src[:, t:t + 1, hf, :])
                ld_wo.append(f)

        def cast_group(pool, lds, g, label, n=8, eng=None, tag=None, bufs=2):
            """bf16 ring tile holding row-tiles [n*g, n*(g+1)) of the tensor."""
            t = pool.tile([P, n, DM], BF16, tag=(tag or label), bufs=bufs,
                          name=f"{label}_bf{g}")
            eng = eng or nc.gpsimd
            for i in range(0, n, 4):
                u = g * n + i
                eng.tensor_copy(out=t[:, i:i + 4, :], in_=lds[u // 4])
            return t

        # k/q path: casts on Pool (key) / DVE (wk, wq, qry); XBARs on the
        # ACT hwdge queue so their waits never stall pending load dispatches
        # keyT2[p, g, t, c] = key[(4g + t//2)*128 + c, (t%2)*128 + p]
        keyT2 = stgK.tile([P, ST // 4, 8, P], BF16)
        for g in range(ST // 4):
            kb = cast_group(stgK, ld_key, g, "key", n=4, bufs=2,
                            eng=(nc.gpsimd if g % 2 == 0 else nc.vector))
            nc.scalar.dma_start_transpose(out=keyT2[:, g], in_=kb)
        wkb_bf = stgK.tile([P, 2, DM], BF16, name="wk_bf")
        nc.vector.tensor_copy(out=wkb_bf, in_=ld_wk[0][:, 0:2, :])
        wkT3 = stgK.tile([P, 4, P], BF16)
        nc.scalar.dma_start_transpose(out=wkT3, in_=wkb_bf)
        wqb_bf = stgK.tile([P, 2, DM], BF16, name="wq_bf")
        nc.vector.tensor_copy(out=wqb_bf, in_=ld_wq[0][:, 0:2, :])
        wqT3 = stgK.tile([P, 4, P], BF16)
        nc.scalar.dma_start_transpose(out=wqT3, in_=wqb_bf)
        qryT2 = stgK.tile([P, QT // 4, 8, P], BF16)
        for g in range(QT // 4):
            qb = cast_group(stgK, ld_qry, g, "qry", n=4, eng=nc.vector)
            nc.scalar.dma_start_transpose(out=qryT2[:, g], in_=qb)
        # wk_b[p, t] = wkb[t*128+p]: channel tile ct gets bias wk_b[:, ct]
        nc.gpsimd.dma_start(out=wk_b, in_=wkb.rearrange("(t p) -> p t", p=P))
        nc.gpsimd.dma_start(out=wq_b, in_=wqb.rearrange("(t p) -> p t", p=P))

        # k proj: psum = sum_dt wkT_dt^T keyT_dt; bf16 out via ACT (+bias)
        mult, add, sub = (mybir.AluOpType.mult, mybir.AluOpType.add,
                          mybir.AluOpType.subtract)
        for ct in range(2):
            for sc in range(ST // 4):
                pp = pP.tile([P, QC], F32, tag="NU", name=f"ppk{ct}_{sc}", bufs=2)
                for dt in range(2):
                    nc.tensor.matmul(
                        pp, wkT3[:, 2 * ct + dt, :],
                        keyT2[:, sc, dt::2, :],
                        start=(dt == 0), stop=(dt == 1))
                nc.scalar.activation(out=kT[:, ct, sc * QC:(sc + 1) * QC],
                                     in_=pp, func=AF.Identity,
                                     bias=wk_b[:, ct:ct + 1])
            for sc in range(QT // 4):
                pp = pP.tile([P, QC], F32, tag="VP", name=f"ppq{ct}_{sc}", bufs=2)
                for dt in range(2):
                    nc.tensor.matmul(
                        pp, wqT3[:, 2 * ct + dt, :],
                        qryT2[:, sc, dt::2, :],
                        start=(dt == 0), stop=(dt == 1))
                nc.scalar.activation(out=qT[:, ct, sc * QC:(sc + 1) * QC],
                                     in_=pp, func=AF.Identity,
                                     bias=wq_b[:, ct:ct + 1])

        # ---- v / wo path: rolling groups, fp8 casts per group ----
        for g in range(ST // 4):
            vb = cast_group(stgV, ld_val, g, "val", n=4, bufs=2)
            nc.sync.dma_start_transpose(out=valT[:, g], in_=vb)
        for g in range(4):
            wvbf = cast_group(stgV, ld_wv, g, "wv", n=4, tag="wvb4")
            nc.sync.dma_start_transpose(out=wvT4[:, g], in_=wvbf)
        for t in range(2):
            for hf in range(2):
                nc.vector.tensor_copy(
                    out=wo_bf[:, t, hf * (H * DM // 2):(hf + 1) * (H * DM // 2)],
                    in_=ld_wo[2 * t + hf][:, 0, :])
        nc.gpsimd.dma_start(out=bv_t, in_=wvb.rearrange("(t p) -> p t", p=P))
        nc.vector.tensor_copy(out=bv_bf, in_=bv_t)
        nc.sync.dma_start(out=wob_row, in_=wob.rearrange("(o c) -> o c", o=1))

    # ---------------- main loop: attention per head pair --------------------
    # late tensors: their pools enter after stage 0, so first-fit places
    # them into the released staging holes (v_ring gates only on the k/q
    # staging teardown, numerT/acc on the v staging teardown)
    mainV = ctx.enter_context(tc.tile_pool(name="mainV", bufs=1))
    v_ring = [mainV.tile([P, ST, 512], BF16, name=f"vring{i}") for i in range(2)]
    mainN = ctx.enter_context(tc.tile_pool(name="mainN", bufs=1))
    numerT = mainN.tile([P, 16, QSH], BF16)      # [c'=(2h+dh)*128+d, q]
    recip = mainN.tile([P, H, QT], F32)          # [q%128, h, q//128] = 1/denom
    acc = mainN.tile([P, QT, DM], F32)           # output accumulator [q, cout]
    mult, add, sub = (mybir.AluOpType.mult, mybir.AluOpType.add,
                      mybir.AluOpType.subtract)
    if True:
        NU_UNITS = H // 2 * NQC  # 8 (pg, qc) units

        def emit_woT():
            """wo head-permuted transposes on the PE."""
            wo_r = wo_bf.rearrange("p t (d h) -> p t h d", h=H)  # [128,2,8,256]
            for kt0 in range(0, 16, 4):
                pt = pP.tile([P, 2 * QC], BF16, tag="NU", name=f"wot{kt0}", bufs=2)
                for i in range(4):
                    kt = kt0 + i
                    h, db = kt // 2, kt % 2
                    for ctt in range(2):
                        nc.tensor.transpose(
                            pt[:, (2 * i + ctt) * P:(2 * i + ctt + 1) * P],
                            wo_r[:, ctt, h, db * P:(db + 1) * P], ident_bf)
                nc.vector.tensor_copy(out=woTp[:, kt0:kt0 + 4, :], in_=pt)
            pb = pP.tile([P, 512], F32, tag="VP", name="pbias", bufs=2)
            for kt in range(16):
                nc.tensor.matmul(pb[0:1, 0:DM], bv_bf[:, kt:kt + 1],
                                 woTp[:, kt, :], start=(kt == 0),
                                 stop=(kt == 15), skip_group_check=True)
            nc.vector.tensor_tensor(out=bo_row, in0=pb[0:1, 0:DM],
                                    in1=wob_row, op=add)
            nc.gpsimd.partition_broadcast(out_ap=wob_bc, in_ap=bo_row)

        def emit_scores_gh(u, g, hh):
            """scores (bf16) + exp for s-tile pair g, head hh of unit u."""
            pg, qc = divmod(u, NQC)
            h = 2 * pg + hh
            base, hg = 32 * (h % 4), h // 4
            E = E_ring[(2 * u + hh) % 5]
            ps = pP.tile([P, 2, QC], F32, tag="SC",
                         name=f"sc{h}_{qc}_{g}", bufs=2)
            for i in range(2):
                st = 2 * g + i
                nc.tensor.matmul(
                    ps[:, i, :],
                    kT[base:base + 32, hg, st * P:(st + 1) * P],
                    qT[base:base + 32, hg, qc * QC:(qc + 1) * QC],
                    start=True, stop=True,
                    tile_position=(base, 0))
            nc.scalar.activation(out=E[:, 2 * g:2 * g + 2, :],
                                 in_=ps, func=AF.Exp)

        def drain_tasks(v):
            """phase-2 work for unit v as a list of small emit-closures."""
            pg, qc = divmod(v, NQC)
            h0 = 2 * pg
            Es = [E_ring[(2 * v + i) % 5] for i in range(2)]
            tasks = []

            def denom(hh):
                def go():
                    h = h0 + hh
                    pdn = pP.tile([P, 512], F32, tag="VP",
                                  name=f"pdn{h}_{qc}", bufs=2)
                    for j in range(4):
                        for st in range(ST):
                            nc.tensor.matmul(
                                pdn[:, j:j + 1],
                                Es[hh][:, st, j * P:(j + 1) * P], ones_bf,
                                start=(st == 0), stop=(st == ST - 1),
                                skip_group_check=True)
                    nc.vector.reciprocal(
                        out=recip[:, h, qc * 4:(qc + 1) * 4], in_=pdn[:, 0:4])
                return go

            def vproj(sp):
                def go():
                    v_pair = v_ring[pg % 2]
                    for st in (2 * sp, 2 * sp + 1):
                        pvt = pP.tile([P, 512], F32, tag="VP",
                                      name=f"pv{pg}_{st}", bufs=2)
                        wvg = wvT4[:, pg]
                        for dt in range(2):
                            wv_rhs = bass.AP(
                                tensor=wvg.tensor,
                                offset=wvg.offset + dt * 128,
                                ap=[list(wvg.ap[0]), [256, 4], [1, 128]])
                            nc.tensor.matmul(
                                pvt,
                                valT[:, st // 4, 2 * (st % 4) + dt, :],
                                wv_rhs, start=(dt == 0), stop=(dt == 1))
                        nc.vector.tensor_copy(out=v_pair[:, st, :], in_=pvt)
                return go

            def numer(hh, dh):
                state = {}

                def first():
                    h = h0 + hh
                    pa = pP.tile([P, QC], F32, tag="NU",
                                 name=f"pa{h}_{qc}_{dh}", bufs=2)
                    state["pa"] = pa
                    v_pair = v_ring[pg % 2]
                    for st in range(ST // 2):
                        nc.tensor.matmul(
                            pa, v_pair[:, st, hh * 256 + dh * P:
                                       hh * 256 + (dh + 1) * P],
                            Es[hh][:, st, :],
                            start=(st == 0), stop=False)

                def second():
                    h = h0 + hh
                    pa = state["pa"]
                    v_pair = v_ring[pg % 2]
                    for st in range(ST // 2, ST):
                        nc.tensor.matmul(
                            pa, v_pair[:, st, hh * 256 + dh * P:
                                       hh * 256 + (dh + 1) * P],
                            Es[hh][:, st, :],
                            start=False, stop=(st == ST - 1))
                    nc.vector.tensor_copy(
                        out=numerT[:, 2 * h + dh, qc * QC:(qc + 1) * QC],
                        in_=pa)
                return first, second

            t0, t1 = [], []
            t0.append(denom(0))
            t1.append(denom(1))
            if qc == 0:
                for sp in range(ST // 2):
                    t0.append(vproj(sp))
            for hh, tl in ((0, t0), (1, t1)):
                for dh in range(2):
                    f, s = numer(hh, dh)
                    tl.append(f)
                    tl.append(s)
            return t0, t1

        def wo_tasks(pg):
            """WO for pair pg, fused with 1/denom and bias accumulation:
            acc[q, :] += (numerT_h^T WOT'_h) * recip_h[q]  (+= bias at h==0)"""
            h0 = 2 * pg

            def one(qt):
                def go():
                    pw = pP.tile([P, 512], F32, tag="NU",
                                 name=f"po{qt}_{pg}", bufs=2)
                    for hh in range(2):
                        h = h0 + hh
                        po = pw[:, hh * DM:(hh + 1) * DM]
                        for dh in range(2):
                            nc.tensor.matmul(
                                po, numerT[:, 2 * h + dh, qt * P:(qt + 1) * P],
                                woTp[:, 2 * h + dh, :],
                                start=(dh == 0), stop=(dh == 1),
                                skip_group_check=True)
                        nc.vector.scalar_tensor_tensor(
                            out=acc[:, qt, :], in0=po,
                            scalar=recip[:, h, qt:qt + 1],
                            in1=(wob_bc if h == 0 else acc[:, qt, :]),
                            op0=mult, op1=add)
                return go
            return [one(qt) for qt in range(QT)]

        # software pipeline, two-unit lag: scores of unit u are emitted in
        # 8 s-pair groups with the drain work of unit u-2 interleaved between
        # them so the PE always has exp-independent work queued behind each
        # ACT-paced score group.  WO is deferred one further unit.
        # E-ring safety: within unit u, ALL tasks reading E(u-2, h0) must be
        # emitted before exp(u, h1) writes that ring slot -- hence the scores
        # run head-major (all h0 groups+tasks, then h1).
        LAG = 2
        pending_wo = []
        for u in range(NU_UNITS + LAG + 1):
            tasks0, tasks1 = [], []
            if u == 2 + LAG:
                tasks1.append(emit_woT)
            if LAG <= u < NU_UNITS + LAG:
                v = u - LAG
                pg, qc = divmod(v, NQC)
                if pending_wo:
                    tasks1.extend(wo_tasks(pending_wo.pop(0)))
                d0, d1 = drain_tasks(v)
                tasks0.extend(d0)
                tasks1.extend(d1)
                if qc == NQC - 1:
                    pending_wo.append(pg)
            if u < NU_UNITS:
                for hh, tasks in ((0, tasks0), (1, tasks1)):
                    nslots = ST // 2
                    ti = 0
                    for g in range(nslots):
                        emit_scores_gh(u, g, hh)
                        upto = (len(tasks) * (g + 1)) // nslots
                        for t in tasks[ti:upto]:
                            t()
                        ti = upto
                    for t in tasks[ti:]:
                        t()
            else:
                for t in tasks0 + tasks1:
                    t()
        for pg in pending_wo:
            for t in wo_tasks(pg):
                t()

        # store the finished output
        for qt in range(QT):
            nc.sync.dma_start(out=out.rearrange("(n p) d -> p n d", p=P)[:, qt, :],
                              in_=acc[:, qt, :])


_NC_CACHE = None


def _get_nc():
    global _NC_CACHE
    if _NC_CACHE is None:
        _NC_CACHE = _build_bass()
    return _NC_CACHE


def _make_in_maps(inputs):
    f = lambda x: np.ascontiguousarray(np.asarray(x, dtype=np.float32))
    shared = {
        "wk": f(inputs["WK_w"]), "wkb": f(inputs["WK_b"]),
        "wq": f(inputs["WQ_w"]), "wqb": f(inputs["WQ_b"]),
        "wv": f(inputs["WV_w"]), "wvb": f(inputs["WV_b"]),
        "wo": f(inputs["WO_w"]), "wob": f(inputs["WO_b"]),
    }
    key_in = f(inputs["key_input"])
    qry_in = f(inputs["query_input"])
    val_in = f(inputs["value_input"])
    in_maps = []
    for c in range(N_CORES):
        b, qs = c // 2, c % 2
        in_maps.append(dict(
            shared,
            key_x=np.ascontiguousarray(key_in[b]),
            qry_x=np.ascontiguousarray(qry_in[b, qs * QSH:(qs + 1) * QSH]),
            val_x=np.ascontiguousarray(val_in[b]),
        ))
    return in_maps


def _assemble(results):
    out = np.empty((B, T2, DM), dtype=np.float32)
    for c in range(N_CORES):
        b, qs = c // 2, c % 2
        out[b, qs * QSH:(qs + 1) * QSH] = results[c]["out_y"]
    return out


def run_spmd(inputs, **kwargs):
    """Run the kernel on all 8 cores; kwargs forwarded (e.g. trace=True)."""
    nc = _get_nc()
    res = run_bass_kernel_spmd(nc, _make_in_maps(inputs),
                               core_ids=list(range(N_CORES)), **kwargs)
    return res


def kernel(**inputs):
    res = run_spmd(inputs)
    return _assemble(res.results)



# revision 3
# speedup vs baseline: 1.5251x; 1.5251x over previous
"""Multi-head attention Bass/Tile kernel for 8 TRN2 NeuronCores.

Problem: nn_MultiHeadAttention (B=4, T1=T2=2048, d_model=256, d_key=32, H=8,
per-head value dim = d_model).  Reference math (no score scaling, no mask):

    k = key   @ WK^T + bk           [B, T1, 256]   (head h -> cols 32h..32h+32)
    q = query @ WQ^T + bq           [B, T2, 256]
    v = value @ WV^T + bv           [B, T1, 2048]  (head h -> cols 256h..256h+256)
    scores_h = k_h q_h^T            [T1, T2]
    attn = softmax over T1 (keys)
    emb_h = attn^T v_h              [T2, 256]
    out = emb' @ WO^T + bo          emb' channel c = d*8 + h (d outer, h inner)

Key algebraic restructure vs the direct form: since softmax weights sum to 1,

    out[q, :] = sum_h (attn_h^T @ val) @ G_h^T + bo
    G_h = WO_h @ WV_h   (host-folded weight product, [256, 256] per head)
    bo  = wob + sum_h WO_h @ bv_h   (host-folded bias)

so the per-head value projection (val @ WV_h^T) and the WO matmul collapse
into a single small per-head GEMM against the unnormalized attention-weighted
value sum P_h[c, q] = sum_s val[s, c] E_h[s, q], normalized at the end by the
per-query 1/denominator.  This removes the v-projection (64 MB intermediate,
~27 us of PE time per core) and all wo transposes from the device program.

Sharding: core c handles (batch b = c//2, query half qs = c%2) -> each core
computes the full output slice out[b, qs*1024:(qs+1)*1024, :].  No collectives.

Layout strategy: the host ships bf16 inputs pre-transposed so the device does
ZERO transposes and ZERO dtype casts:
  - keyT/qryT [256, s] channel-major (for the k/q projections)
  - val [s, 256] natural (stationary operand of the P matmul contracts over s)
  - gt [h*256+c, e] = G_h^T rows (rhs of the output GEMM)

Per-core pipeline (units u = (head, 512-query-chunk), 16 units):
  - scores_h = kT_h^T qT_h on PE into PSUM [s, q] tiles (bf16, tile_position
    packs the 32-row stationary into PE quadrant rows 32*(h%4)).
  - E = exp(scores) on ACT (no max subtraction; fp32 range is plenty), the
    only ACT work in the kernel so the Exp table is loaded exactly once.
  - P(u): 32 chained matmuls val-stationary x E -> psum [c, 512q], one unit
    behind the scores; denominators via ones-matmul (out free size 1).
  - copies P->SBUF bf16 + reciprocal on DVE, two units behind.
  - out2(h): psum [q, e] = P_h^T G_h^T, scaled by recip and accumulated into
    acc on DVE (scalar_tensor_tensor), bias folded in at h==0.
  - k/q projections for the second head group are deferred into unit 0's
    task list so the PE never idles waiting for DMA at startup.

kernel(**inputs) takes the FULL unsharded inputs and returns the full output.
"""

import numpy as np
import ml_dtypes
from contextlib import ExitStack

import concourse.bass as bass
import concourse.bacc as bacc
import concourse.mybir as mybir
import concourse.tile as tile
from concourse.bass_utils import run_bass_kernel_spmd

P = 128
B, T1, T2, DM, DK, H = 4, 2048, 2048, 256, 32, 8
QSH = T2 // 2  # queries per core
N_CORES = 8

F32 = mybir.dt.float32
BF16 = mybir.dt.bfloat16
AF = mybir.ActivationFunctionType
BF = ml_dtypes.bfloat16

ST = T1 // P        # 16 key/seq tiles
QT = QSH // P       # 8 query tiles per core
QC = 512            # query chunk (PSUM free dim)
NQC = QSH // QC     # 2 query chunks
NU = H * NQC        # 16 pipeline units


def _build_bass():
    nc = bacc.Bacc("TRN2", target_bir_lowering=False, debug=False)

    keyT = nc.dram_tensor("keyT", [DM, T1], BF16, kind="ExternalInput").ap()
    qryT = nc.dram_tensor("qryT", [DM, QSH], BF16, kind="ExternalInput").ap()
    val = nc.dram_tensor("val_x", [T1, DM], BF16, kind="ExternalInput").ap()
    wkT = nc.dram_tensor("wkT", [DM, DM], BF16, kind="ExternalInput").ap()
    wqT = nc.dram_tensor("wqT", [DM, DM], BF16, kind="ExternalInput").ap()
    wkb = nc.dram_tensor("wkb", [DM], F32, kind="ExternalInput").ap()
    wqb = nc.dram_tensor("wqb", [DM], F32, kind="ExternalInput").ap()
    gt = nc.dram_tensor("gt", [H * DM, DM], BF16, kind="ExternalInput").ap()
    bo = nc.dram_tensor("bo", [DM], F32, kind="ExternalInput").ap()
    out = nc.dram_tensor("out_y", [QSH, DM], F32, kind="ExternalOutput").ap()

    with tile.TileContext(nc, pool_alloc_mode="queue") as tc:
        with ExitStack() as ctx:
            _body(ctx, tc, keyT, qryT, val, wkT, wqT, wkb, wqb, gt, bo, out)
    nc.compile()
    return nc


def _body(ctx, tc, keyT, qryT, val, wkT, wqT, wkb, wqb, gt, bo, out):
    nc = tc.nc
    mult, add = mybir.AluOpType.mult, mybir.AluOpType.add
    consts = ctx.enter_context(tc.tile_pool(name="consts", bufs=1))
    main = ctx.enter_context(tc.tile_pool(name="main", bufs=1))
    # One PSUM pool, 8 banks total:
    #   SC: [128,2,512] f32 x2  (4 banks) score tiles
    #   PP: [128,512]  f32 x2   (2 banks) k-proj, then P accumulators
    #   OU: [128,512]  f32 x2   (2 banks) q-proj, denominators, out2 psums
    pP = ctx.enter_context(tc.tile_pool(name="pP", bufs=1, space="PSUM"))

    # touch the ACT Exp table early; exp is the ONLY ACT op in the kernel so
    # the table is loaded exactly once, off the critical path
    actwarm = consts.tile([1, 1], F32)
    nc.scalar.activation(out=actwarm, in_=actwarm, func=AF.Exp)
    ones_bf = consts.tile([P, 1], BF16)
    nc.vector.memset(ones_bf, 1.0)
    wk_b = consts.tile([P, 2], F32)
    wq_b = consts.tile([P, 2], F32)
    bo_row = consts.tile([1, DM], F32)
    bo_bc = consts.tile([P, DM], F32)
    # biases on the gpsimd (SWDGE) queue: off the shared HWDGE device
    nc.gpsimd.dma_start(out=wk_b, in_=wkb.rearrange("(t p) -> p t", p=P))
    nc.gpsimd.dma_start(out=wq_b, in_=wqb.rearrange("(t p) -> p t", p=P))
    nc.gpsimd.dma_start(out=bo_row, in_=bo.rearrange("(o c) -> o c", o=1))
    nc.gpsimd.partition_broadcast(out_ap=bo_bc, in_ap=bo_row)

    # persistent SBUF tensors
    keyT_sb = main.tile([P, 2, T1], BF16)        # [c%128, c//128, s]
    qryT_sb = main.tile([P, 2, QSH], BF16)
    wkT_sb = main.tile([P, 2, DM], BF16)         # [c%128, c//128, ch]
    wqT_sb = main.tile([P, 2, DM], BF16)
    val_sb = main.tile([P, ST, DM], BF16)        # [s%128, s//128, c]
    gt_sb = main.tile([P, 2 * H, DM], BF16)      # [c%128, 2h+(c//128), e]
    kT = main.tile([P, 2, T1], BF16)             # [32*(h%4)+d, h//4, s]
    qT = main.tile([P, 2, QSH], BF16)
    E_ring = [main.tile([P, ST, QC], BF16, name=f"Er{i}") for i in range(2)]
    P_sb = [main.tile([P, 2, QSH], BF16, name=f"Psb{i}") for i in range(2)]
    recip_t = main.tile([P, H, QT], F32)         # [q%128, h, q//128] = 1/denom
    acc = main.tile([P, QT, DM], F32)            # output accumulator [q, e]

    # ---------------- DMA loads, priority-ordered ---------------------------
    keyT_src = keyT.rearrange("(t p) s -> p t s", p=P)
    qryT_src = qryT.rearrange("(t p) s -> p t s", p=P)
    val_src = val.rearrange("(n p) c -> p n c", p=P)
    gt_src = gt.rearrange("(n p) e -> p n e", p=P)
    nc.sync.dma_start(out=wkT_sb, in_=wkT.rearrange("(t p) c -> p t c", p=P))
    for sc in range(4):
        nc.sync.dma_start(out=keyT_sb[:, :, sc * QC:(sc + 1) * QC],
                          in_=keyT_src[:, :, sc * QC:(sc + 1) * QC])
    nc.scalar.dma_start(out=wqT_sb, in_=wqT.rearrange("(t p) c -> p t c", p=P))
    for sc in range(2):
        nc.scalar.dma_start(out=qryT_sb[:, :, sc * QC:(sc + 1) * QC],
                            in_=qryT_src[:, :, sc * QC:(sc + 1) * QC])
    for i in range(2):
        nc.sync.dma_start(out=val_sb[:, i * 8:(i + 1) * 8, :],
                          in_=val_src[:, i * 8:(i + 1) * 8, :])
    for i in range(2):
        nc.scalar.dma_start(out=gt_sb[:, i * 8:(i + 1) * 8, :],
                            in_=gt_src[:, i * 8:(i + 1) * 8, :])

    # ---------------- k/q projections (copies on DVE, keeping ACT exp-only) -
    def kproj(ct, sc):
        pp = pP.tile([P, QC], F32, tag="PP", bufs=2, name=f"ppk{ct}_{sc}")
        for t in range(2):
            nc.tensor.matmul(pp, wkT_sb[:, t, ct * P:(ct + 1) * P],
                             keyT_sb[:, t, sc * QC:(sc + 1) * QC],
                             start=(t == 0), stop=(t == 1))
        nc.vector.tensor_scalar_add(out=kT[:, ct, sc * QC:(sc + 1) * QC],
                                    in0=pp, scalar1=wk_b[:, ct:ct + 1])

    def qproj(ct, sc):
        pq = pP.tile([P, QC], F32, tag="OU", bufs=2, name=f"ppq{ct}_{sc}")
        for t in range(2):
            nc.tensor.matmul(pq, wqT_sb[:, t, ct * P:(ct + 1) * P],
                             qryT_sb[:, t, sc * QC:(sc + 1) * QC],
                             start=(t == 0), stop=(t == 1))
        nc.vector.tensor_scalar_add(out=qT[:, ct, sc * QC:(sc + 1) * QC],
                                    in0=pq, scalar1=wq_b[:, ct:ct + 1])

    # minimal projections for unit 0 (head 0, q-chunk 0); the rest are
    # deferred into unit 0's task list (kT/qT ct covers heads 4ct..4ct+3)
    kproj(0, 0)
    qproj(0, 0)

    # ---------------- main pipeline -----------------------------------------
    out_r = out.rearrange("(n p) d -> p n d", p=P)
    Pp = {}  # unit v -> [P psum ct0, ct1]
    Dn = {}  # unit v -> denominator psum

    def emit_scores(u, g):
        h, qc = divmod(u, NQC)
        base, hg = 32 * (h % 4), h // 4
        E = E_ring[u % 2]
        ps = pP.tile([P, 2, QC], F32, tag="SC", bufs=2, name=f"sc{u}_{g}")
        for i in range(2):
            st = 2 * g + i
            nc.tensor.matmul(
                ps[:, i, :],
                kT[base:base + 32, hg, st * P:(st + 1) * P],
                qT[base:base + 32, hg, qc * QC:(qc + 1) * QC],
                start=True, stop=True,
                tile_position=(base, 0))
        nc.scalar.activation(out=E[:, 2 * g:2 * g + 2, :], in_=ps, func=AF.Exp)

    def build_tasks(u):
        T = []
        # DVE: copies + reciprocal for unit u-2 (its P/denom psums completed
        # at the end of unit u-1); frees the PP ring for this unit's P chain
        if 2 <= u <= NU + 1:
            v = u - 2
            h, qc = divmod(v, NQC)

            def cps(v=v, h=h, qc=qc):
                for ct in range(2):
                    nc.vector.tensor_copy(
                        out=P_sb[h % 2][:, ct, qc * QC:(qc + 1) * QC],
                        in_=Pp[v][ct])
                nc.vector.reciprocal(out=recip_t[:, h, qc * 4:(qc + 1) * 4],
                                     in_=Dn[v][:, 0:4])
            T.append(cps)

            # out2 for head h on the query half that was just copied
            def out2_pair(qp, h=h):
                po = pP.tile([P, 2, DM], F32, tag="OU", bufs=2,
                             name=f"o2_{h}_{qp}")
                for i in range(2):
                    qt = 2 * qp + i
                    for ct in range(2):
                        nc.tensor.matmul(
                            po[:, i, :],
                            P_sb[h % 2][:, ct, qt * P:(qt + 1) * P],
                            gt_sb[:, 2 * h + ct, :],
                            start=(ct == 0), stop=(ct == 1),
                            skip_group_check=True)
                for i in range(2):
                    qt = 2 * qp + i
                    nc.vector.scalar_tensor_tensor(
                        out=acc[:, qt, :], in0=po[:, i, :],
                        scalar=recip_t[:, h, qt:qt + 1],
                        in1=(bo_bc if h == 0 else acc[:, qt, :]),
                        op0=mult, op1=add)
                    if h == H - 1 and qt % 2 == 1:
                        nc.sync.dma_start(out=out_r[:, qt - 1:qt + 1, :],
                                          in_=acc[:, qt - 1:qt + 1, :])
            for qp in ((0, 1) if qc == 0 else (2, 3)):
                T.append(lambda qp=qp: out2_pair(qp))

        # PE: P chain + denominators for unit u-1 (E(u-1) is complete)
        if 1 <= u <= NU:
            v = u - 1
            E = E_ring[v % 2]

            def pchain(g, v=v, E=E):
                if g == 0:
                    Pp[v] = [pP.tile([P, QC], F32, tag="PP", bufs=2,
                                     name=f"pp{v}_{ct}") for ct in range(2)]
                for i in range(2):
                    st = 2 * g + i
                    for ct in range(2):
                        nc.tensor.matmul(
                            Pp[v][ct], val_sb[:, st, ct * P:(ct + 1) * P],
                            E[:, st, :],
                            start=(st == 0), stop=(st == ST - 1),
                            skip_group_check=True)

            def dchain(j, v=v, E=E):
                if j == 0:
                    Dn[v] = pP.tile([P, 512], F32, tag="OU", bufs=2,
                                    name=f"dn{v}")
                for st in range(ST):
                    nc.tensor.matmul(
                        Dn[v][:, j:j + 1], E[:, st, j * P:(j + 1) * P],
                        ones_bf,
                        start=(st == 0), stop=(st == ST - 1),
                        skip_group_check=True)

            # P-chain early (E tiles ready one-by-one), denominators late
            # (they read all 16 s-tiles of E)
            for g in range(8):
                T.append(lambda g=g: pchain(g))
                if g >= 4:
                    T.append(lambda j=g - 4: dchain(j))

        # deferred projections: head group 1 + remaining chunks (unit 0 only)
        if u == 0:
            for ct, sc in ((0, 1), (0, 2), (0, 3), (1, 0), (1, 1), (1, 2),
                           (1, 3)):
                T.append(lambda ct=ct, sc=sc: kproj(ct, sc))
            for ct, sc in ((0, 1), (1, 0), (1, 1)):
                T.append(lambda ct=ct, sc=sc: qproj(ct, sc))
        return T

    for u in range(NU + 2):
        T = build_tasks(u)
        if u < NU:
            ti = 0
            for g in range(8):
                emit_scores(u, g)
                upto = (len(T) * (g + 1)) // 8
                for t in T[ti:upto]:
                    t()
                ti = upto
            for t in T[ti:]:
                t()
        else:
            for t in T:
                t()


_NC_CACHE = None


def _get_nc():
    global _NC_CACHE
    if _NC_CACHE is None:
        _NC_CACHE = _build_bass()
    return _NC_CACHE


def _make_in_maps(inputs):
    f = lambda x: np.asarray(x, dtype=np.float32)
    WK, WKb = f(inputs["WK_w"]), f(inputs["WK_b"])
    WQ, WQb = f(inputs["WQ_w"]), f(inputs["WQ_b"])
    WV, WVb = f(inputs["WV_w"]), f(inputs["WV_b"])
    WO, WOb = f(inputs["WO_w"]), f(inputs["WO_b"])

    # host-folded weights: G_h = WO_h @ WV_h, shipped as gt[h*256+c, e] = G_h^T
    WO_r = WO.reshape(DM, DM, H)                  # [e, dm, h]
    gt = np.empty((H, DM, DM), np.float32)        # [h, c, e]
    for h in range(H):
        gt[h] = (WO_r[:, :, h] @ WV[h * DM:(h + 1) * DM]).T
    gt = np.ascontiguousarray(gt.reshape(H * DM, DM).astype(BF))
    # host-folded bias: bo = wob + sum_h WO_h @ bv_h  (softmax rows sum to 1)
    bvec = WVb.reshape(H, DM).T.reshape(-1)       # [dm*8 + h]
    bo = np.ascontiguousarray((WOb + WO @ bvec).astype(np.float32))

    shared = {
        "wkT": np.ascontiguousarray(WK.T.astype(BF)),
        "wqT": np.ascontiguousarray(WQ.T.astype(BF)),
        "wkb": np.ascontiguousarray(WKb),
        "wqb": np.ascontiguousarray(WQb),
        "gt": gt,
        "bo": bo,
    }
    key_in = f(inputs["key_input"])
    qry_in = f(inputs["query_input"])
    val_in = f(inputs["value_input"])
    in_maps = []
    for c in range(N_CORES):
        b, qs = c // 2, c % 2
        in_maps.append(dict(
            shared,
            keyT=np.ascontiguousarray(key_in[b].T.astype(BF)),
            qryT=np.ascontiguousarray(
                qry_in[b, qs * QSH:(qs + 1) * QSH].T.astype(BF)),
            val_x=np.ascontiguousarray(val_in[b].astype(BF)),
        ))
    return in_maps


def _assemble(results):
    out = np.empty((B, T2, DM), dtype=np.float32)
    for c in range(N_CORES):
        b, qs = c // 2, c % 2
        out[b, qs * QSH:(qs + 1) * QSH] = results[c]["out_y"]
    return out


def run_spmd(inputs, **kwargs):
    """Run the kernel on all 8 cores; kwargs forwarded (e.g. trace=True)."""
    nc = _get_nc()
    res = run_bass_kernel_spmd(nc, _make_in_maps(inputs),
                               core_ids=list(range(N_CORES)), **kwargs)
    return res


def kernel(**inputs):
    res = run_spmd(inputs)
    return _assemble(res.results)
